# revision 1
# baseline (speedup 1.0000x reference)
"""BiDAF attention kernel for 8 Trainium2 NeuronCores (data-parallel over batch).

Contract: kernel(**inputs) takes the FULL unsharded inputs (as produced by the
reference setup_inputs) and returns the FULL [16, 1024, 2048] fp32 output.

Math (per batch b):
    s[i,j]  = c[i].c_w + q[j].q_w + sum_h c[i,h]*cqw[h]*q[j,h] + bias
    s1      = softmax_j(masked(s, q_mask));  s2 = softmax_i(masked(s, c_mask))
    a       = s1 @ q ; bb = s1 @ s2^T @ c
    out     = concat(c, a, c*a, c*bb)

Device mapping (per core: 2 batches):
  - Host folds cq_weight and c_weight into the q side:  qw'[j,h] = q*cqw + c_w
    so one PE matmul chain gives sT[j,i] = sim_cq[i,j] + sim_c[i].
  - sim_q + bias + q_mask fold into the Exp activation's per-partition bias.
  - c_mask (when non-trivial) is added via a K=1 rank-1 matmul.
  - Softmax without max-subtraction (values bounded, fp32-safe): one exp(sT)
    serves both softmaxes; normalizations are per-partition scales applied to
    the downstream matmul outputs.
  - t = s2T@c via PE-transposed exp(sT); a/b via e as stationary operand.
  - The c block of the output is assembled on the host (pure copy of an
    input); the device emits only the computed a | c*a | c*b blocks.

Precision modes (BIDAF_DTYPE): "mixed" (default; similarity chain fp32,
post-softmax matmuls float32r), "fp32" (all exact), "fp32r" (all reduced).
"""

import os
import sys
from contextlib import ExitStack

import numpy as np

for _p in ("/opt/trn_rl_repo", "/root/.axon_site/_ro/trn_rl_repo"):
    if os.path.isdir(_p) and _p not in sys.path:
        sys.path.append(_p)

B, CL, QL, H = 16, 1024, 128, 512
N_CORES = 8
BPC = B // N_CORES  # batches per core
NEG = np.float32(-1e30)

DTYPE_MODE = os.environ.get("BIDAF_DTYPE", "fp32r")

_build_cache = {}


def _build(mask_trivial: bool, mode: str):
    key = (mask_trivial, mode)
    if key in _build_cache:
        return _build_cache[key]

    import concourse.bass as bass
    import concourse.tile as tile
    from concourse import bacc, mybir

    F32 = mybir.dt.float32
    F32R = mybir.dt.float32r
    SIM_DT = F32R if mode == "fp32r" else F32  # similarity-chain matmul dtype
    DOWN_DT = F32 if mode == "fp32" else F32R  # post-softmax matmul dtype
    AF = mybir.ActivationFunctionType
    PSUM = bass.MemorySpace.PSUM

    nc = bacc.Bacc("TRN2", target_bir_lowering=False, debug=False)

    c_d = nc.dram_tensor("c", [BPC, CL, H], F32, kind="ExternalInput")
    # qpack[:, 0:512] = qw'.T tiles, [:, 512] = qbias, [:, 513:1025] = q
    qpack_d = nc.dram_tensor("qpack", [BPC, 128, 1025], F32, kind="ExternalInput")
    ident_d = nc.dram_tensor("ident", [128, 128], F32, kind="ExternalInput")
    if not mask_trivial:
        cmask_d = nc.dram_tensor("cmaskb", [BPC, 1, CL], F32, kind="ExternalInput")
        onesr_d = nc.dram_tensor("onesr", [1, QL], F32, kind="ExternalInput")
    onesc_d = nc.dram_tensor("onesc", [QL, 1], F32, kind="ExternalInput")
    aca_d = nc.dram_tensor("out_aca", [BPC, CL, 2 * H], F32, kind="ExternalOutput")
    cb_d = nc.dram_tensor("out_cb", [BPC, CL, H], F32, kind="ExternalOutput")

    KT = H // 128  # 4 k-tiles over the hidden dim
    IT = CL // 128  # 8 i-tiles over the context dim

    with tile.TileContext(nc) as tc, ExitStack() as ctx:
        const = ctx.enter_context(tc.tile_pool(name="const", bufs=1))
        sbp = ctx.enter_context(tc.tile_pool(name="sbp", bufs=2))
        outp = ctx.enter_context(tc.tile_pool(name="outp", bufs=6))
        ps_acc = ctx.enter_context(tc.tile_pool(name="ps_acc", bufs=2, space=PSUM))
        ps_tr = ctx.enter_context(tc.tile_pool(name="ps_tr", bufs=3, space=PSUM))
        ps_ab = ctx.enter_context(tc.tile_pool(name="ps_ab", bufs=3, space=PSUM))

        ident = const.tile([128, 128], F32, tag="ident")
        nc.sync.dma_start(ident[:], ident_d.ap())
        if DOWN_DT != F32:
            ident_e = const.tile([128, 128], DOWN_DT, tag="ident_e")
            nc.vector.tensor_copy(ident_e[:], ident[:])
        else:
            ident_e = ident
        if not mask_trivial:
            cmask_f = const.tile([1, BPC * CL], F32, tag="cmask_f")
            nc.sync.dma_start(cmask_f[:], cmask_d.ap().rearrange("b one i -> one (b i)"))
            onesr_f = const.tile([1, QL], F32, tag="onesr_f")
            nc.sync.dma_start(onesr_f[:], onesr_d.ap())
            if SIM_DT != F32:
                cmask_all = const.tile([1, BPC * CL], SIM_DT, tag="cmask")
                nc.vector.tensor_copy(cmask_all[:], cmask_f[:])
                onesr = const.tile([1, QL], SIM_DT, tag="onesr")
                nc.vector.tensor_copy(onesr[:], onesr_f[:])
            else:
                cmask_all, onesr = cmask_f, onesr_f

        onesc_f = const.tile([QL, 1], F32, tag="onesc_f")
        nc.sync.dma_start(onesc_f[:], onesc_d.ap())

        # ---- PE clock warmup + ACT exp-table preload in the preamble window:
        # memsets go on DVE so they are not queued behind the Q7/SWDGE
        # descriptor generation of the cast c loads (that delay previously
        # pushed the warmup past 14us and left the PE cold-clocked through
        # the whole similarity front end).
        BF16 = mybir.dt.bfloat16
        warmf = const.tile([128, 1], F32, tag="warmf")
        nc.vector.memset(warmf[:], 0.0)
        nc.scalar.activation(warmf[:, 0:1], warmf[:, 0:1], AF.Exp)
        warmL = const.tile([128, 1], BF16, tag="warmL")
        warmC = const.tile([128, 512], BF16, tag="warmC")
        nc.vector.memset(warmL[:], 0.0)
        nc.vector.memset(warmC[:], 0.0)
        pw = ps_tr.tile([128, 512], F32, tag="tr")
        for _ in range(8):
            nc.tensor.matmul(pw[:1, :], warmL[:], warmC[:], start=True, stop=True)

        # ---- phase A: emit ALL loads (both batches). c loads cast to the
        # matmul dtype in-flight (SWDGE); everything else on the SP HWDGE
        # queue. CDT values feed transposes/traw/cr1; the output's exact c
        # block is assembled host-side, so rounding c here only perturbs the
        # already-approximate a/ca/cb products.
        CDT = DOWN_DT if mode == "fp32r" else F32
        LD = []
        for bi in range(BPC):
            ch = []
            for qt in range(4):
                cht = sbp.tile([128, 2, H], CDT, tag=f"cq{qt}")
                src = c_d.ap()[bi, qt * 256 : (qt + 1) * 256, :].rearrange(
                    "(t p) h -> p t h", p=128
                )
                if CDT != F32:
                    nc.gpsimd.dma_start(cht[:], src)
                else:
                    nc.sync.dma_start(cht[:], src)
                ch.append(cht)
            qpk = sbp.tile([128, 1025], F32, tag="qpk")
            nc.sync.dma_start(qpk[:], qpack_d.ap()[bi])
            LD.append((ch, qpk))

        # ---- phase B1: per-batch similarity front-end + a|c*a stores ----
        ST = []
        for bi in range(BPC):
            ch, qpk = LD[bi]
            c_sb = [ch[it // 2][:, it % 2, :] for it in range(IT)]
            qbias_sb = qpk[:, 512:513]
            if SIM_DT != F32:
                qwT_sb = sbp.tile([128, KT, QL], SIM_DT, tag="qwT_sb")
                nc.vector.tensor_copy(
                    qwT_sb[:], qpk[:, 0:512].rearrange("p (t j) -> p t j", t=KT)
                )
            else:
                qwT_sb = qpk[:, 0:512].rearrange("p (t j) -> p t j", t=KT)
            if DOWN_DT != F32:
                q_sb = sbp.tile([QL, H], DOWN_DT, tag="q_sb")
                nc.vector.tensor_copy(q_sb[:], qpk[:, 513:1025])
            else:
                q_sb = qpk[:, 513:1025]

            # ---- per CL-half front end: cT transposes, sT, exp, colsums,
            #      then the a | c*a column stores (independent of the b path)
            ehalf = []
            r1h = []
            cr1s = []
            rs2 = sbp.tile([QL, 2], F32, tag="rs2")
            for nh in range(2):
                cth = sbp.tile([128, KT, 512], SIM_DT, tag=f"ct{nh}")
                for j in range(4):
                    it = 4 * nh + j
                    ptr = ps_tr.tile([128, 512], CDT, tag="tr")
                    for k in range(KT):
                        nc.tensor.transpose(
                            ptr[:, k * 128 : (k + 1) * 128],
                            c_sb[it][:, k * 128 : (k + 1) * 128],
                            ident_e[:] if CDT != F32 else ident[:],
                        )
                    src = ptr[:].rearrange("p (k i) -> p k i", k=KT)
                    if j % 2 == 0:
                        nc.vector.tensor_copy(cth[:, :, j * 128 : (j + 1) * 128], src)
                    else:
                        nc.scalar.copy(cth[:, :, j * 128 : (j + 1) * 128], src)

                spt = ps_acc.tile([QL, 512], F32, tag="acc")
                for k in range(KT):
                    nc.tensor.matmul(
                        spt[:],
                        qwT_sb[:, k, :],
                        cth[:, k, :],
                        start=(k == 0),
                        stop=(k == KT - 1 and mask_trivial),
                    )
                if not mask_trivial:
                    nc.tensor.matmul(
                        spt[:],
                        onesr[:],
                        cmask_all[:, bi * CL + nh * 512 : bi * CL + (nh + 1) * 512],
                        start=False,
                        stop=True,
                    )

                eh = sbp.tile([QL, 512], DOWN_DT, tag=f"e{nh}")
                nc.scalar.activation(
                    eh[:],
                    spt[:],
                    AF.Exp,
                    bias=qbias_sb[:],
                    scale=1.0,
                    accum_out=rs2[:, nh : nh + 1],
                )
                ehalf.append(eh)

                pcs = ps_ab.tile([128, 4], F32, tag="ab")
                for j in range(4):
                    nc.tensor.matmul(
                        pcs[:, j : j + 1],
                        eh[:, j * 128 : (j + 1) * 128].bitcast(F32),
                        onesc_f[:],
                        start=True,
                        stop=True,
                    )
                r1n = sbp.tile([128, 4], F32, tag=f"r1{nh}")
                nc.vector.reciprocal(r1n[:], pcs[:])
                r1h.append(r1n)

                # a | c*a for this half's i-tiles (cr1 = r1*c folds the
                # softmax normalization into the c-products)
                for j in range(4):
                    it = 4 * nh + j
                    esl = eh[:, j * 128 : (j + 1) * 128]
                    pa = ps_ab.tile([128, H], F32, tag="ab")
                    nc.tensor.matmul(pa[:], esl, q_sb[:], start=True, stop=True)
                    cr1 = sbp.tile([128, H], F32, tag=f"cr1_{it}")
                    nc.vector.tensor_scalar_mul(cr1[:], c_sb[it], r1n[:, j : j + 1])
                    cr1s.append(cr1)
                    aca_sb = outp.tile([128, 2 * H], F32, tag="aca")
                    nc.scalar.mul(aca_sb[:, 0:H], pa[:], r1n[:, j : j + 1])
                    nc.vector.tensor_mul(aca_sb[:, H : 2 * H], cr1[:], pa[:])
                    rows = aca_d.ap()[bi, it * 128 : (it + 1) * 128]
                    nc.sync.dma_start(rows[:], aca_sb[:])

            ST.append((c_sb, ehalf, r1h, cr1s, rs2))

        # ---- phase B2: per-batch b path: r2, eN, traw, t, c*b stores ----
        for bi in range(BPC):
            c_sb, ehalf, r1h, cr1s, rs2 = ST[bi]
            rsum = sbp.tile([QL, 1], F32, tag="rsum")
            nc.vector.tensor_reduce(rsum[:], rs2[:], mybir.AxisListType.X, mybir.AluOpType.add)
            r2 = sbp.tile([QL, 1], F32, tag="r2")
            nc.vector.reciprocal(r2[:], rsum[:])

            eN = []
            for half in range(2):
                pe = ps_tr.tile([128, 512], DOWN_DT, tag="tr")
                for j in range(4):
                    nc.tensor.transpose(
                        pe[:, j * 128 : (j + 1) * 128],
                        ehalf[half][:, j * 128 : (j + 1) * 128],
                        ident_e[:],
                    )
                eNh = sbp.tile([128, 4, 128], CDT, tag=f"eN{half}")
                if half == 0:
                    nc.vector.tensor_copy(eNh[:], pe[:].rearrange("p (t j) -> p t j", t=4))
                else:
                    nc.scalar.copy(eNh[:], pe[:].rearrange("p (t j) -> p t j", t=4))
                eN.append(eNh)

            ptraw = ps_acc.tile([QL, H], F32, tag="acc")
            for it in range(IT):
                nc.tensor.matmul(
                    ptraw[:],
                    eN[it // 4][:, it % 4, :],
                    c_sb[it],
                    start=(it == 0),
                    stop=(it == IT - 1),
                )
            t_sb = sbp.tile([QL, H], DOWN_DT, tag="t")
            nc.scalar.mul(t_sb[:], ptraw[:], r2[:])

            for it in range(IT):
                esl = ehalf[it // 4][:, (it % 4) * 128 : (it % 4 + 1) * 128]
                pb = ps_ab.tile([128, H], F32, tag="ab")
                nc.tensor.matmul(pb[:], esl, t_sb[:], start=True, stop=True)
                cb_sb = outp.tile([128, H], F32, tag="cb")
                nc.vector.tensor_mul(cb_sb[:], cr1s[it][:], pb[:])
                rows = cb_d.ap()[bi, it * 128 : (it + 1) * 128]
                nc.sync.dma_start(rows[:], cb_sb[:])

    nc.compile()
    _build_cache[key] = nc
    return nc


def _install_profshim():
    """Optional NTFF profiling support (BIDAF_PROFILE=1); self-contained."""
    import contextlib
    import ctypes
    import types

    if "antenv.axon_hooks" in sys.modules:
        return
    so_path = "/opt/axon/libaxon_pjrt.so"
    try:
        lib = ctypes.CDLL(so_path)
    except OSError:
        return
    if not hasattr(lib, "axon_start_nrt_profile"):
        return
    lib.axon_start_nrt_profile.argtypes = [ctypes.POINTER(ctypes.c_int64), ctypes.c_size_t]
    lib.axon_start_nrt_profile.restype = ctypes.c_int64
    lib.axon_stop_nrt_profile.argtypes = [ctypes.c_char_p]
    lib.axon_stop_nrt_profile.restype = ctypes.c_int64

    @contextlib.contextmanager
    def _hook(output_dir, device_ids):
        import jax

        jax.devices()
        if device_ids:
            ids = (ctypes.c_int64 * len(device_ids))(*device_ids)
            rc = lib.axon_start_nrt_profile(ids, len(device_ids))
        else:
            rc = lib.axon_start_nrt_profile(None, 0)
        if rc != 0:
            raise RuntimeError(f"axon_start_nrt_profile rc={rc}")
        try:
            yield
        finally:
            n = lib.axon_stop_nrt_profile(str(output_dir).encode())
            print(f"profile: {n} file(s) written to {output_dir}")

    mod = types.ModuleType("antenv.axon_hooks")
    mod.get_axon_ntff_profile_hook = lambda: _hook
    mod.set_axon_ntff_profile_hook = lambda h: None
    sys.modules["antenv.axon_hooks"] = mod
    import antenv

    antenv.axon_hooks = mod

    from concourse import bass_utils

    bass_utils.upload_artifacts = lambda tmpdir: f"local:{tmpdir}"


def kernel(c, q, c_mask, q_mask, c_weight, q_weight, cq_weight, bias):
    from concourse.bass_utils import run_bass_kernel_spmd

    c = np.asarray(c, dtype=np.float32)
    q = np.asarray(q, dtype=np.float32)
    c_mask = np.asarray(c_mask)
    q_mask = np.asarray(q_mask)
    c_weight = np.asarray(c_weight, dtype=np.float32)
    q_weight = np.asarray(q_weight, dtype=np.float32)
    cq_weight = np.asarray(cq_weight, dtype=np.float32)
    bias = np.asarray(bias, dtype=np.float32)

    # host-side folding (all tiny, O(B*(CL+QL)*H) at most)
    qw = q * cq_weight.reshape(1, 1, H) + c_weight.reshape(1, 1, H)  # [B, QL, H]
    sim_q = (q @ q_weight)[:, :, 0]  # [B, QL]
    amask_q = (1.0 - q_mask.astype(np.float32)) * NEG
    qbias = (sim_q + bias[0] + amask_q).astype(np.float32)  # [B, QL]
    amask_c = ((1.0 - c_mask.astype(np.float32)) * NEG).reshape(B, 1, CL)
    mask_trivial = bool((amask_c == 0).all())
    KT = H // 128
    qpack = np.empty((B, 128, 1025), dtype=np.float32)
    qpack[:, :, 0:512] = (
        qw.reshape(B, QL, KT, 128).transpose(0, 3, 2, 1).reshape(B, 128, KT * QL)
    )
    qpack[:, :, 512] = qbias
    qpack[:, :, 513:1025] = q

    profile = os.environ.get("BIDAF_PROFILE", "") == "1"
    if profile:
        _install_profshim()

    nc = _build(mask_trivial, DTYPE_MODE)

    ident = np.eye(128, dtype=np.float32)
    onesr = np.ones((1, QL), dtype=np.float32)
    in_maps = []
    for core in range(N_CORES):
        s = slice(BPC * core, BPC * (core + 1))
        m = {
            "c": np.ascontiguousarray(c[s]),
            "qpack": np.ascontiguousarray(qpack[s]),
            "ident": ident,
            "onesc": np.ones((QL, 1), dtype=np.float32),
        }
        if not mask_trivial:
            m["cmaskb"] = np.ascontiguousarray(amask_c[s])
            m["onesr"] = onesr
        in_maps.append(m)

    kw = {}
    if profile:
        kw = dict(trace=True, tmpdir=os.environ.get("BIDAF_PROFILE_DIR") or None)
    res = run_bass_kernel_spmd(nc, in_maps, list(range(N_CORES)), **kw)
    if profile and res.exec_time_ns is not None:
        print(f"[kernel] HW exec time: {res.exec_time_ns} ns")
        kernel.last_exec_time_ns = res.exec_time_ns
        kernel.last_trace = res.instructions_and_trace[1] if res.instructions_and_trace else None

    out = np.empty((B, CL, 4 * H), dtype=np.float32)
    out[:, :, 0:H] = c
    for i in range(N_CORES):
        out[BPC * i : BPC * (i + 1), :, H : 3 * H] = res.results[i]["out_aca"]
        out[BPC * i : BPC * (i + 1), :, 3 * H :] = res.results[i]["out_cb"]
    return out


kernel.last_exec_time_ns = None
kernel.last_trace = None



# revision 9
# speedup vs baseline: 1.0312x; 1.0312x over previous
"""BiDAF attention kernel for 8 Trainium2 NeuronCores (data-parallel over batch).

Contract: kernel(**inputs) takes the FULL unsharded inputs (as produced by the
reference setup_inputs) and returns the FULL [16, 1024, 2048] fp32 output.

Math (per batch b):
    s[i,j]  = c[i].c_w + q[j].q_w + sum_h c[i,h]*cqw[h]*q[j,h] + bias
    s1      = softmax_j(masked(s, q_mask));  s2 = softmax_i(masked(s, c_mask))
    a       = s1 @ q ; bb = s1 @ s2^T @ c
    out     = concat(c, a, c*a, c*bb)

Device mapping (per core: 2 batches):
  - Host folds cq_weight and c_weight into the q side:  qw'[j,h] = q*cqw + c_w
    so one PE matmul chain gives sT[j,i] = sim_cq[i,j] + sim_c[i].
  - sim_q + bias + q_mask fold into the Exp activation's per-partition bias.
  - All HBM loads are plain HWDGE fp32 transfers; fp32r is obtained by
    bitcasting APs at the matmul operands (identical bits, no cast DMA and no
    SWDGE descriptor generation).
  - Softmax without max-subtraction (values bounded, fp32-safe): one exp(sT)
    serves both softmaxes.  The s1 normalizer r1 (column sums of e) falls out
    of the eN-transpose PSUM->SBUF copies via ACT accum_out, so no rank-1
    column-sum matmuls are needed.  r2 comes from the exp's own accum_out.
  - a-block: ACT applies r1 (a = pa*r1), DVE does ca = c .* a with all-SBUF
    operands (2x DVE mode).  b-block: one DVE scalar_tensor_tensor computes
    (pb * r1) .* c straight out of PSUM.
  - Outputs stream out in large (1-2 MB) HWDGE stores; the exact c block of
    the output is assembled host-side (pure copy of an input).
"""

import os
import sys
from contextlib import ExitStack

import numpy as np

for _p in ("/opt/trn_rl_repo", "/root/.axon_site/_ro/trn_rl_repo"):
    if os.path.isdir(_p) and _p not in sys.path:
        sys.path.append(_p)

B, CL, QL, H = 16, 1024, 128, 512
N_CORES = 8
BPC = B // N_CORES  # batches per core
NEG = np.float32(-1e30)

KT = H // 128  # 4 k-tiles over the hidden dim
IT = CL // 128  # 8 i-tiles over the context dim

_build_cache = {}


def _build(mask_trivial: bool):
    key = mask_trivial
    if key in _build_cache:
        return _build_cache[key]

    import concourse.bass as bass
    import concourse.tile as tile
    from concourse import bacc, mybir

    F32 = mybir.dt.float32
    F32R = mybir.dt.float32r
    AF = mybir.ActivationFunctionType
    MUL = mybir.AluOpType.mult
    PSUM = bass.MemorySpace.PSUM

    nc = bacc.Bacc("TRN2", target_bir_lowering=False, debug=False)

    # Inputs are declared float32r (same bit layout as fp32): plain HWDGE
    # DMAs satisfy the BIR verifier's fp32r-producer rule with no cast pass.
    c_d = nc.dram_tensor("c", [BPC, CL, H], F32R, kind="ExternalInput")
    # qpack[:, 0:512] = qw'.T tiles, [:, 512] = qbias, [:, 513:1025] = q
    qpack_d = nc.dram_tensor("qpack", [BPC, 128, 1025], F32R, kind="ExternalInput")
    ident_d = nc.dram_tensor("ident", [128, 128], F32R, kind="ExternalInput")
    if not mask_trivial:
        cmask_d = nc.dram_tensor("cmaskb", [BPC, 1, CL], F32R, kind="ExternalInput")
        onesr_d = nc.dram_tensor("onesr", [1, QL], F32R, kind="ExternalInput")
    aca_d = nc.dram_tensor("out_aca", [BPC, CL, 2 * H], F32, kind="ExternalOutput")
    cb_d = nc.dram_tensor("out_cb", [BPC, CL, H], F32, kind="ExternalOutput")

    with tile.TileContext(nc) as tc, ExitStack() as ctx:
        const = ctx.enter_context(tc.tile_pool(name="const", bufs=1))
        sbp = ctx.enter_context(tc.tile_pool(name="sbp", bufs=2))
        outp = ctx.enter_context(tc.tile_pool(name="outp", bufs=2))
        ps_acc = ctx.enter_context(tc.tile_pool(name="ps_acc", bufs=2, space=PSUM))
        ps_tr = ctx.enter_context(tc.tile_pool(name="ps_tr", bufs=3, space=PSUM))
        ps_ab = ctx.enter_context(tc.tile_pool(name="ps_ab", bufs=3, space=PSUM))

        # ---- phase A: all HBM loads up front, HWDGE only. First c half-batch
        # leads so the transpose front-end can start as soon as possible.
        LD = []
        for bi in range(BPC):
            ch = []
            for nh in range(2):
                cht = sbp.tile([128, 4, H], F32R, tag=f"c{nh}")
                src = c_d.ap()[bi, nh * 512 : (nh + 1) * 512, :].rearrange(
                    "(t p) h -> p t h", p=128
                )
                nc.sync.dma_start(cht[:], src)
                ch.append(cht)
                if bi == 0 and nh == 0:
                    ident = const.tile([128, 128], F32R, tag="ident")
                    nc.sync.dma_start(ident[:], ident_d.ap())
                    identr = ident[:]
            qpk = sbp.tile([128, 1025], F32R, tag="qp")
            nc.sync.dma_start(qpk[:], qpack_d.ap()[bi])
            LD.append((ch, qpk))
        if not mask_trivial:
            cmask_f = const.tile([1, BPC * CL], F32R, tag="cmask_f")
            nc.sync.dma_start(cmask_f[:], cmask_d.ap().rearrange("b one i -> one (b i)"))
            onesr_f = const.tile([1, QL], F32R, tag="onesr_f")
            nc.sync.dma_start(onesr_f[:], onesr_d.ap())

        # ---- PE clock warmup + ACT exp-table preload while loads stream.
        BF16 = mybir.dt.bfloat16
        warmf = const.tile([128, 1], F32, tag="warmf")
        nc.vector.memset(warmf[:], 0.0)
        nc.scalar.activation(warmf[:, 0:1], warmf[:, 0:1], AF.Exp)
        warmL = const.tile([128, 1], BF16, tag="warmL")
        warmC = const.tile([128, 512], BF16, tag="warmC")
        nc.vector.memset(warmL[:], 0.0)
        nc.vector.memset(warmC[:], 0.0)
        pw = ps_tr.tile([128, 512], F32, tag="tr")
        for _ in range(8):
            nc.tensor.matmul(pw[:1, :], warmL[:], warmC[:], start=True, stop=True)

        # ---- per-batch pipeline ----
        for bi in range(BPC):
            ch, qpk = LD[bi]
            c_sb = [ch[it // 4][:, it % 4, :] for it in range(IT)]
            qbias_sb = qpk[:, 512:513].bitcast(F32)
            qwT = qpk[:, 0:512].rearrange("p (t j) -> p t j", t=KT)
            q_sb = qpk[:, 513:1025]

            rs2 = sbp.tile([QL, 2], F32, tag="rs2")
            ehalf, eNs, r1h = [], [], []

            # -- phase B1 per CL-half: cT transposes, sT, exp, eN (+r1 via
            #    accum), then the a | c*a stores
            for nh in range(2):
                cth = sbp.tile([128, KT, 512], F32R, tag="cth")
                for j in range(4):
                    it = 4 * nh + j
                    ptr = ps_tr.tile([128, 512], F32R, tag="tr")
                    for k in range(KT):
                        nc.tensor.transpose(
                            ptr[:, k * 128 : (k + 1) * 128],
                            c_sb[it][:, k * 128 : (k + 1) * 128],
                            identr,
                        )
                    src = ptr[:].rearrange("p (k i) -> p k i", k=KT)
                    if j % 2 == 0:
                        nc.vector.tensor_copy(cth[:, :, j * 128 : (j + 1) * 128], src)
                    else:
                        nc.scalar.copy(cth[:, :, j * 128 : (j + 1) * 128], src)

                spt = ps_acc.tile([QL, 512], F32, tag="acc")
                for k in range(KT):
                    nc.tensor.matmul(
                        spt[:],
                        qwT[:, k, :],
                        cth[:, k, :],
                        start=(k == 0),
                        stop=(k == KT - 1 and mask_trivial),
                    )
                if not mask_trivial:
                    nc.tensor.matmul(
                        spt[:],
                        onesr_f[:],
                        cmask_f[:, bi * CL + nh * 512 : bi * CL + (nh + 1) * 512],
                        start=False,
                        stop=True,
                    )

                eh = sbp.tile([QL, 512], F32R, tag=f"e{nh}")
                nc.scalar.activation(
                    eh[:],
                    spt[:],
                    AF.Exp,
                    bias=qbias_sb[:],
                    scale=1.0,
                    accum_out=rs2[:, nh : nh + 1],
                )
                ehalf.append(eh)

                # eN = e^T per j-block; the PSUM->SBUF copies also emit the
                # column sums of e (accum_out) = the s1 normalizers.
                pe = ps_tr.tile([128, 512], F32R, tag="tr")
                for j in range(4):
                    nc.tensor.transpose(
                        pe[:, j * 128 : (j + 1) * 128],
                        eh[:, j * 128 : (j + 1) * 128],
                        identr,
                    )
                eNh = sbp.tile([128, 4, 128], F32R, tag=f"eN{nh}")
                csum = sbp.tile([128, 4], F32, tag=f"cs{nh}")
                for j in range(4):
                    nc.scalar.activation(
                        eNh[:, j, :],
                        pe[:, j * 128 : (j + 1) * 128],
                        AF.Copy,
                        accum_out=csum[:, j : j + 1],
                    )
                eNs.append(eNh)
                r1n = sbp.tile([128, 4], F32, tag=f"r1{nh}")
                nc.vector.reciprocal(r1n[:], csum[:])
                r1h.append(r1n)

                # a | c*a for this half's i-tiles
                aca_sb = outp.tile([128, 4, 2 * H], F32, tag="aca")
                for j in range(4):
                    it = 4 * nh + j
                    esl = eh[:, j * 128 : (j + 1) * 128]
                    pa = ps_ab.tile([128, H], F32, tag="ab")
                    nc.tensor.matmul(pa[:], esl, q_sb, start=True, stop=True)
                    nc.scalar.mul(aca_sb[:, j, 0:H], pa[:], r1n[:, j : j + 1])
                    nc.vector.tensor_mul(
                        aca_sb[:, j, H : 2 * H], c_sb[it], aca_sb[:, j, 0:H]
                    )
                    if j % 2 == 1:
                        rows = aca_d.ap()[
                            bi, nh * 512 + (j - 1) * 128 : nh * 512 + (j + 1) * 128
                        ].rearrange("(t p) h -> p t h", p=128)
                        nc.sync.dma_start(rows[:], aca_sb[:, j - 1 : j + 1, :])

            # -- phase B2: r2, t = (s2^T c) * r2, then b path and c*b stores
            rsum = sbp.tile([QL, 1], F32, tag="rsum")
            nc.vector.tensor_reduce(
                rsum[:], rs2[:], mybir.AxisListType.X, mybir.AluOpType.add
            )
            r2 = sbp.tile([QL, 1], F32, tag="r2")
            nc.vector.reciprocal(r2[:], rsum[:])

            ptraw = ps_acc.tile([QL, H], F32, tag="acc")
            for it in range(IT):
                nc.tensor.matmul(
                    ptraw[:],
                    eNs[it // 4][:, it % 4, :],
                    c_sb[it],
                    start=(it == 0),
                    stop=(it == IT - 1),
                )
            t_sb = sbp.tile([QL, H], F32R, tag="t")
            nc.scalar.mul(t_sb[:], ptraw[:], r2[:])

            for nh in range(2):
                cb_sb = outp.tile([128, 4, H], F32, tag="cb")
                for j in range(4):
                    it = 4 * nh + j
                    esl = ehalf[nh][:, j * 128 : (j + 1) * 128]
                    pb = ps_ab.tile([128, H], F32, tag="ab")
                    nc.tensor.matmul(pb[:], esl, t_sb[:], start=True, stop=True)
                    nc.vector.scalar_tensor_tensor(
                        cb_sb[:, j, :],
                        pb[:],
                        r1h[nh][:, j : j + 1],
                        c_sb[it],
                        MUL,
                        MUL,
                    )
                rows = cb_d.ap()[bi, nh * 512 : (nh + 1) * 512].rearrange(
                    "(t p) h -> p t h", p=128
                )
                nc.sync.dma_start(rows[:], cb_sb[:])

    nc.compile()
    _build_cache[key] = nc
    return nc


def _install_profshim():
    """Optional NTFF profiling support (BIDAF_PROFILE=1); self-contained."""
    import contextlib
    import ctypes
    import types

    if "antenv.axon_hooks" in sys.modules:
        return
    so_path = "/opt/axon/libaxon_pjrt.so"
    try:
        lib = ctypes.CDLL(so_path)
    except OSError:
        return
    if not hasattr(lib, "axon_start_nrt_profile"):
        return
    lib.axon_start_nrt_profile.argtypes = [ctypes.POINTER(ctypes.c_int64), ctypes.c_size_t]
    lib.axon_start_nrt_profile.restype = ctypes.c_int64
    lib.axon_stop_nrt_profile.argtypes = [ctypes.c_char_p]
    lib.axon_stop_nrt_profile.restype = ctypes.c_int64

    @contextlib.contextmanager
    def _hook(output_dir, device_ids):
        import jax

        jax.devices()
        if device_ids:
            ids = (ctypes.c_int64 * len(device_ids))(*device_ids)
            rc = lib.axon_start_nrt_profile(ids, len(device_ids))
        else:
            rc = lib.axon_start_nrt_profile(None, 0)
        if rc != 0:
            raise RuntimeError(f"axon_start_nrt_profile rc={rc}")
        try:
            yield
        finally:
            n = lib.axon_stop_nrt_profile(str(output_dir).encode())
            print(f"profile: {n} file(s) written to {output_dir}")

    mod = types.ModuleType("antenv.axon_hooks")
    mod.get_axon_ntff_profile_hook = lambda: _hook
    mod.set_axon_ntff_profile_hook = lambda h: None
    sys.modules["antenv.axon_hooks"] = mod
    import antenv

    antenv.axon_hooks = mod

    from concourse import bass_utils

    bass_utils.upload_artifacts = lambda tmpdir: f"local:{tmpdir}"


def kernel(c, q, c_mask, q_mask, c_weight, q_weight, cq_weight, bias):
    from concourse.bass_utils import run_bass_kernel_spmd

    c = np.asarray(c, dtype=np.float32)
    q = np.asarray(q, dtype=np.float32)
    c_mask = np.asarray(c_mask)
    q_mask = np.asarray(q_mask)
    c_weight = np.asarray(c_weight, dtype=np.float32)
    q_weight = np.asarray(q_weight, dtype=np.float32)
    cq_weight = np.asarray(cq_weight, dtype=np.float32)
    bias = np.asarray(bias, dtype=np.float32)

    # host-side folding (all tiny, O(B*(CL+QL)*H) at most)
    qw = q * cq_weight.reshape(1, 1, H) + c_weight.reshape(1, 1, H)  # [B, QL, H]
    sim_q = (q @ q_weight)[:, :, 0]  # [B, QL]
    amask_q = (1.0 - q_mask.astype(np.float32)) * NEG
    qbias = (sim_q + bias[0] + amask_q).astype(np.float32)  # [B, QL]
    amask_c = ((1.0 - c_mask.astype(np.float32)) * NEG).reshape(B, 1, CL)
    mask_trivial = bool((amask_c == 0).all())
    qpack = np.empty((B, 128, 1025), dtype=np.float32)
    qpack[:, :, 0:512] = (
        qw.reshape(B, QL, KT, 128).transpose(0, 3, 2, 1).reshape(B, 128, KT * QL)
    )
    qpack[:, :, 512] = qbias
    qpack[:, :, 513:1025] = q

    profile = os.environ.get("BIDAF_PROFILE", "") == "1"
    if profile:
        _install_profshim()

    nc = _build(mask_trivial)

    ident = np.eye(128, dtype=np.float32)
    in_maps = []
    for core in range(N_CORES):
        s = slice(BPC * core, BPC * (core + 1))
        m = {
            "c": np.ascontiguousarray(c[s]),
            "qpack": np.ascontiguousarray(qpack[s]),
            "ident": ident,
        }
        if not mask_trivial:
            m["cmaskb"] = np.ascontiguousarray(amask_c[s])
            m["onesr"] = np.ones((1, QL), dtype=np.float32)
        in_maps.append(m)

    kw = {}
    if profile:
        kw = dict(trace=True, tmpdir=os.environ.get("BIDAF_PROFILE_DIR") or None)
    res = run_bass_kernel_spmd(nc, in_maps, list(range(N_CORES)), **kw)
    if profile and res.exec_time_ns is not None:
        print(f"[kernel] HW exec time: {res.exec_time_ns} ns")
        kernel.last_exec_time_ns = res.exec_time_ns
        kernel.last_trace = res.instructions_and_trace[1] if res.instructions_and_trace else None

    out = np.empty((B, CL, 4 * H), dtype=np.float32)
    out[:, :, 0:H] = c
    for i in range(N_CORES):
        out[BPC * i : BPC * (i + 1), :, H : 3 * H] = res.results[i]["out_aca"]
        out[BPC * i : BPC * (i + 1), :, 3 * H :] = res.results[i]["out_cb"]
    return out


kernel.last_exec_time_ns = None
kernel.last_trace = None


# revision 11
# speedup vs baseline: 1.2002x; 1.1638x over previous
"""BiDAF attention kernel for 8 Trainium2 NeuronCores (data-parallel over batch).

Contract: kernel(**inputs) takes the FULL unsharded inputs (as produced by the
reference setup_inputs) and returns the FULL [16, 1024, 2048] fp32 output.

Math (per batch b):
    s[i,j]  = c[i].c_w + q[j].q_w + sum_h c[i,h]*cqw[h]*q[j,h] + bias
    s1      = softmax_j(masked(s, q_mask));  s2 = softmax_i(masked(s, c_mask))
    a       = s1 @ q ; bb = s1 @ s2^T @ c
    out     = concat(c, a, c*a, c*bb)

Device mapping (per core: 2 batches), v3 = bf16 matmul pipeline:
  - Host folds cq_weight and c_weight into the q side (qw' = q*cqw + c_w) and
    ships bf16 copies of c in BOTH layouts: c[i,h] and cT[h,i].  That removes
    all on-device context transposes and their PSUM round-trips; the
    similarity chain reads cT straight from HBM.  sim_q + bias + q_mask fold
    into the Exp activation's per-partition fp32 bias.
  - All matmuls run in bf16 (error budget: rel tol is 2e-2, bf16 chain lands
    ~1e-3): sT = qw'T.T @ cT; exp -> e (bf16); a = (e@q)*r1; t = (eN@c)*r2;
    b = (e@t)*r1.
  - One exp serves both softmaxes (no max-subtraction; values bounded).
    r2 row-sums fall out of the Exp accum_out; r1 column-sums fall out of a
    single DVE 3D reduce over the e-transpose PSUM tile.
  - c*a runs alternately on DVE and GpSimd (both SBUF-only operands);
    c*b is one DVE scalar_tensor_tensor (pb * r1) * c straight out of PSUM.
  - Outputs are fp32; they stream out in 0.5-1 MB HWDGE stores as soon as
    each i-tile pair is ready.  The exact c block of the output is assembled
    host-side (pure copy of an input).
"""

import os
import sys
from contextlib import ExitStack

import numpy as np
import ml_dtypes

for _p in ("/opt/trn_rl_repo", "/root/.axon_site/_ro/trn_rl_repo"):
    if os.path.isdir(_p) and _p not in sys.path:
        sys.path.append(_p)

B, CL, QL, H = 16, 1024, 128, 512
N_CORES = 8
BPC = B // N_CORES  # batches per core
NEG = np.float32(-1e30)
BF = ml_dtypes.bfloat16

KT = H // 128  # 4 k-tiles over the hidden dim
IT = CL // 128  # 8 i-tiles over the context dim

_build_cache = {}


def _build(mask_trivial: bool):
    key = mask_trivial
    if key in _build_cache:
        return _build_cache[key]

    import concourse.bass as bass
    import concourse.tile as tile
    from concourse import bacc, mybir

    F32 = mybir.dt.float32
    BF16 = mybir.dt.bfloat16
    AF = mybir.ActivationFunctionType
    MUL = mybir.AluOpType.mult
    PSUM = bass.MemorySpace.PSUM

    nc = bacc.Bacc("TRN2", target_bir_lowering=False, debug=False)

    cbf_d = nc.dram_tensor("cbf", [BPC, CL, H], BF16, kind="ExternalInput")
    ctb_d = nc.dram_tensor("ctb", [BPC, H, CL], BF16, kind="ExternalInput")
    # qpk[:, 0:512] = qw'.T k-tiles (bf16), [:, 512:1024] = q (bf16)
    qpk_d = nc.dram_tensor("qpk", [BPC, 128, 1024], BF16, kind="ExternalInput")
    qb_d = nc.dram_tensor("qb", [128, BPC], F32, kind="ExternalInput")
    ident_d = nc.dram_tensor("ident", [128, 128], BF16, kind="ExternalInput")
    if not mask_trivial:
        cmask_d = nc.dram_tensor("cmaskb", [BPC, 1, CL], BF16, kind="ExternalInput")
        onesr_d = nc.dram_tensor("onesr", [1, QL], BF16, kind="ExternalInput")
    aca_d = nc.dram_tensor("out_aca", [BPC, CL, 2 * H], F32, kind="ExternalOutput")
    cb_d = nc.dram_tensor("out_cb", [BPC, CL, H], F32, kind="ExternalOutput")

    with tile.TileContext(nc) as tc, ExitStack() as ctx:
        const = ctx.enter_context(tc.tile_pool(name="const", bufs=1))
        sbp = ctx.enter_context(tc.tile_pool(name="sbp", bufs=2))
        outp = ctx.enter_context(tc.tile_pool(name="outp", bufs=2))
        ps_acc = ctx.enter_context(tc.tile_pool(name="ps_acc", bufs=2, space=PSUM))
        ps_tr = ctx.enter_context(tc.tile_pool(name="ps_tr", bufs=2, space=PSUM))
        ps_ab = ctx.enter_context(tc.tile_pool(name="ps_ab", bufs=4, space=PSUM))

        # ---- phase A: all HBM loads up front, HWDGE only; first the slices
        # the front end needs soonest.
        LD = []
        for bi in range(BPC):
            ctb = sbp.tile([128, KT, CL], BF16, tag="ct")
            for ih in range(2):
                src = ctb_d.ap()[bi, :, ih * 512 : (ih + 1) * 512].rearrange(
                    "(t p) i -> p t i", p=128
                )
                nc.sync.dma_start(ctb[:, :, ih * 512 : (ih + 1) * 512], src)
                if bi == 0 and ih == 0:
                    qpk = sbp.tile([128, 1024], BF16, tag="qp")
                    nc.sync.dma_start(qpk[:], qpk_d.ap()[bi])
                    qb = const.tile([128, BPC], F32, tag="qb")
                    nc.sync.dma_start(qb[:], qb_d.ap())
                    ident = const.tile([128, 128], BF16, tag="ident")
                    nc.sync.dma_start(ident[:], ident_d.ap())
            if bi > 0:
                qpk = sbp.tile([128, 1024], BF16, tag="qp")
                nc.sync.dma_start(qpk[:], qpk_d.ap()[bi])
            ch = []
            for nh in range(2):
                cht = sbp.tile([128, 4, H], BF16, tag=f"c{nh}")
                src = cbf_d.ap()[bi, nh * 512 : (nh + 1) * 512, :].rearrange(
                    "(t p) h -> p t h", p=128
                )
                nc.sync.dma_start(cht[:], src)
                ch.append(cht)
            LD.append((ctb, ch, qpk))
        if not mask_trivial:
            cmask_f = const.tile([1, BPC * CL], BF16, tag="cmask_f")
            nc.sync.dma_start(cmask_f[:], cmask_d.ap().rearrange("b one i -> one (b i)"))
            onesr_f = const.tile([1, QL], BF16, tag="onesr_f")
            nc.sync.dma_start(onesr_f[:], onesr_d.ap())

        # ---- PE clock warmup + ACT exp-table preload while loads stream.
        warmf = const.tile([128, 1], F32, tag="warmf")
        nc.vector.memset(warmf[:], 0.0)
        nc.scalar.activation(warmf[:, 0:1], warmf[:, 0:1], AF.Exp)
        warmL = const.tile([128, 1], BF16, tag="warmL")
        warmC = const.tile([128, 512], BF16, tag="warmC")
        nc.vector.memset(warmL[:], 0.0)
        nc.vector.memset(warmC[:], 0.0)
        pw = ps_acc.tile([QL, 512], F32, tag="acc")
        for _ in range(5):
            nc.tensor.matmul(pw[:1, :], warmL[:], warmC[:], start=True, stop=True)

        # ---- per-batch pipeline ----
        for bi in range(BPC):
            ctb, ch, qpk = LD[bi]
            c_sb = [ch[it // 4][:, it % 4, :] for it in range(IT)]
            qwT = qpk[:, 0:512].rearrange("p (t j) -> p t j", t=KT)
            q_sb = qpk[:, 512:1024]
            qbias = qb[:, bi : bi + 1]

            rs2 = sbp.tile([QL, 2], F32, tag="rs2")
            ehalf, eNs, r1h = [], [], []

            # -- phase B1 per CL-half: sT, exp, eN (+r1 via DVE reduce),
            #    then the a | c*a stores
            for nh in range(2):
                spt = ps_acc.tile([QL, 512], F32, tag="acc")
                for k in range(KT):
                    nc.tensor.matmul(
                        spt[:],
                        qwT[:, k, :],
                        ctb[:, k, nh * 512 : (nh + 1) * 512],
                        start=(k == 0),
                        stop=(k == KT - 1 and mask_trivial),
                    )
                if not mask_trivial:
                    nc.tensor.matmul(
                        spt[:],
                        onesr_f[:],
                        cmask_f[:, bi * CL + nh * 512 : bi * CL + (nh + 1) * 512],
                        start=False,
                        stop=True,
                    )

                eh = sbp.tile([QL, 512], BF16, tag=f"e{nh}")
                nc.scalar.activation(
                    eh[:],
                    spt[:],
                    AF.Exp,
                    bias=qbias,
                    scale=1.0,
                    accum_out=rs2[:, nh : nh + 1],
                )
                ehalf.append(eh)

                # eN = e^T per j-block; r1 (s1 normalizers) via one DVE
                # 3D reduce over the transpose PSUM tile.
                pe = ps_tr.tile([128, 512], BF16, tag="tr")
                for j in range(4):
                    nc.tensor.transpose(
                        pe[:, j * 128 : (j + 1) * 128],
                        eh[:, j * 128 : (j + 1) * 128],
                        ident[:],
                    )
                csum = sbp.tile([128, 4], F32, tag=f"cs{nh}")
                nc.vector.tensor_reduce(
                    csum[:],
                    pe[:].rearrange("p (j q) -> p j q", j=4),
                    mybir.AxisListType.X,
                    mybir.AluOpType.add,
                )
                r1n = sbp.tile([128, 4], F32, tag=f"r1{nh}")
                nc.vector.reciprocal(r1n[:], csum[:])
                r1h.append(r1n)
                eNh = sbp.tile([128, 4, 128], BF16, tag=f"eN{nh}")
                nc.scalar.copy(eNh[:], pe[:].rearrange("p (j q) -> p j q", j=4))
                eNs.append(eNh)

                # a | c*a for this half's i-tiles
                aca_sb = outp.tile([128, 4, 2 * H], F32, tag="aca")
                for j in range(4):
                    it = 4 * nh + j
                    esl = eh[:, j * 128 : (j + 1) * 128]
                    pa = ps_ab.tile([128, H], F32, tag="ab")
                    nc.tensor.matmul(pa[:], esl, q_sb, start=True, stop=True)
                    nc.scalar.mul(aca_sb[:, j, 0:H], pa[:], r1n[:, j : j + 1])
                    if j % 2 == 0:
                        nc.vector.tensor_mul(
                            aca_sb[:, j, H : 2 * H], c_sb[it], aca_sb[:, j, 0:H]
                        )
                    else:
                        nc.gpsimd.tensor_mul(
                            aca_sb[:, j, H : 2 * H], c_sb[it], aca_sb[:, j, 0:H]
                        )
                    if j % 2 == 1:
                        rows = aca_d.ap()[
                            bi, nh * 512 + (j - 1) * 128 : nh * 512 + (j + 1) * 128
                        ].rearrange("(t p) h -> p t h", p=128)
                        nc.sync.dma_start(rows[:], aca_sb[:, j - 1 : j + 1, :])

            # -- phase B2: r2, t = (s2^T c) * r2, then b path and c*b stores
            rsum = sbp.tile([QL, 1], F32, tag="rsum")
            nc.vector.tensor_reduce(
                rsum[:], rs2[:], mybir.AxisListType.X, mybir.AluOpType.add
            )
            r2 = sbp.tile([QL, 1], F32, tag="r2")
            nc.vector.reciprocal(r2[:], rsum[:])

            ptraw = ps_acc.tile([QL, H], F32, tag="acc")
            for it in range(IT):
                nc.tensor.matmul(
                    ptraw[:],
                    eNs[it // 4][:, it % 4, :],
                    c_sb[it],
                    start=(it == 0),
                    stop=(it == IT - 1),
                )
            t_sb = sbp.tile([QL, H], BF16, tag="t")
            nc.scalar.mul(t_sb[:], ptraw[:], r2[:])

            for nh in range(2):
                cb_sb = outp.tile([128, 4, H], F32, tag="cb")
                for j in range(4):
                    it = 4 * nh + j
                    esl = ehalf[nh][:, j * 128 : (j + 1) * 128]
                    pb = ps_ab.tile([128, H], F32, tag="ab")
                    nc.tensor.matmul(pb[:], esl, t_sb[:], start=True, stop=True)
                    nc.vector.scalar_tensor_tensor(
                        cb_sb[:, j, :],
                        pb[:],
                        r1h[nh][:, j : j + 1],
                        c_sb[it],
                        MUL,
                        MUL,
                    )
                    if j % 2 == 1:
                        rows = cb_d.ap()[
                            bi, nh * 512 + (j - 1) * 128 : nh * 512 + (j + 1) * 128
                        ].rearrange("(t p) h -> p t h", p=128)
                        nc.sync.dma_start(rows[:], cb_sb[:, j - 1 : j + 1, :])

    nc.compile()
    _build_cache[key] = nc
    return nc


def _install_profshim():
    """Optional NTFF profiling support (BIDAF_PROFILE=1); self-contained."""
    import contextlib
    import ctypes
    import types

    if "antenv.axon_hooks" in sys.modules:
        return
    so_path = "/opt/axon/libaxon_pjrt.so"
    try:
        lib = ctypes.CDLL(so_path)
    except OSError:
        return
    if not hasattr(lib, "axon_start_nrt_profile"):
        return
    lib.axon_start_nrt_profile.argtypes = [ctypes.POINTER(ctypes.c_int64), ctypes.c_size_t]
    lib.axon_start_nrt_profile.restype = ctypes.c_int64
    lib.axon_stop_nrt_profile.argtypes = [ctypes.c_char_p]
    lib.axon_stop_nrt_profile.restype = ctypes.c_int64

    @contextlib.contextmanager
    def _hook(output_dir, device_ids):
        import jax

        jax.devices()
        if device_ids:
            ids = (ctypes.c_int64 * len(device_ids))(*device_ids)
            rc = lib.axon_start_nrt_profile(ids, len(device_ids))
        else:
            rc = lib.axon_start_nrt_profile(None, 0)
        if rc != 0:
            raise RuntimeError(f"axon_start_nrt_profile rc={rc}")
        try:
            yield
        finally:
            n = lib.axon_stop_nrt_profile(str(output_dir).encode())
            print(f"profile: {n} file(s) written to {output_dir}")

    mod = types.ModuleType("antenv.axon_hooks")
    mod.get_axon_ntff_profile_hook = lambda: _hook
    mod.set_axon_ntff_profile_hook = lambda h: None
    sys.modules["antenv.axon_hooks"] = mod
    import antenv

    antenv.axon_hooks = mod

    from concourse import bass_utils

    bass_utils.upload_artifacts = lambda tmpdir: f"local:{tmpdir}"


def kernel(c, q, c_mask, q_mask, c_weight, q_weight, cq_weight, bias):
    from concourse.bass_utils import run_bass_kernel_spmd

    c = np.asarray(c, dtype=np.float32)
    q = np.asarray(q, dtype=np.float32)
    c_mask = np.asarray(c_mask)
    q_mask = np.asarray(q_mask)
    c_weight = np.asarray(c_weight, dtype=np.float32)
    q_weight = np.asarray(q_weight, dtype=np.float32)
    cq_weight = np.asarray(cq_weight, dtype=np.float32)
    bias = np.asarray(bias, dtype=np.float32)

    # host-side folding + bf16 input marshalling
    qw = q * cq_weight.reshape(1, 1, H) + c_weight.reshape(1, 1, H)  # [B, QL, H]
    sim_q = (q @ q_weight)[:, :, 0]  # [B, QL]
    amask_q = (1.0 - q_mask.astype(np.float32)) * NEG
    qbias = (sim_q + bias[0] + amask_q).astype(np.float32)  # [B, QL]
    amask_c = ((1.0 - c_mask.astype(np.float32)) * NEG).reshape(B, 1, CL)
    mask_trivial = bool((amask_c == 0).all())

    cbf = c.astype(BF)  # [B, CL, H]
    ctb = np.ascontiguousarray(cbf.transpose(0, 2, 1))  # [B, H, CL]
    qpk = np.empty((B, 128, 1024), dtype=BF)
    qpk[:, :, 0:512] = (
        qw.reshape(B, QL, KT, 128).transpose(0, 3, 2, 1).reshape(B, 128, KT * QL)
    ).astype(BF)
    qpk[:, :, 512:1024] = q.astype(BF)

    profile = os.environ.get("BIDAF_PROFILE", "") == "1"
    if profile:
        _install_profshim()

    nc = _build(mask_trivial)

    ident = np.eye(128, dtype=BF)
    in_maps = []
    for core in range(N_CORES):
        s = slice(BPC * core, BPC * (core + 1))
        m = {
            "cbf": np.ascontiguousarray(cbf[s]),
            "ctb": np.ascontiguousarray(ctb[s]),
            "qpk": np.ascontiguousarray(qpk[s]),
            "qb": np.ascontiguousarray(qbias[s].T),
            "ident": ident,
        }
        if not mask_trivial:
            m["cmaskb"] = np.ascontiguousarray(amask_c[s]).astype(BF)
            m["onesr"] = np.ones((1, QL), dtype=BF)
        in_maps.append(m)

    kw = {}
    if profile:
        kw = dict(trace=True, tmpdir=os.environ.get("BIDAF_PROFILE_DIR") or None)
    res = run_bass_kernel_spmd(nc, in_maps, list(range(N_CORES)), **kw)
    if profile and res.exec_time_ns is not None:
        print(f"[kernel] HW exec time: {res.exec_time_ns} ns")
        kernel.last_exec_time_ns = res.exec_time_ns
        kernel.last_trace = res.instructions_and_trace[1] if res.instructions_and_trace else None

    out = np.empty((B, CL, 4 * H), dtype=np.float32)
    out[:, :, 0:H] = c
    for i in range(N_CORES):
        out[BPC * i : BPC * (i + 1), :, H : 3 * H] = res.results[i]["out_aca"]
        out[BPC * i : BPC * (i + 1), :, 3 * H :] = res.results[i]["out_cb"]
    return out


kernel.last_exec_time_ns = None
kernel.last_trace = None
